# revision 1
# baseline (speedup 1.0000x reference)
"""DigitCapsuleLayer (dynamic routing) Trainium2 Bass kernel.

Sharding: P-parallel — the 1152 primary capsules are split 144-per-core
across 8 cores; every core holds the full batch B=128 on SBUF partitions.

Per core:
  phase 1 (TensorE): per p, u_hat[b, od] = x_p[8,128].T @ W_p[8,160]
    (K=8 contraction over in_dim), evacuated 3-p-at-a-time from PSUM into
    an SBUF-resident u_hat [128, 144*160].  A second accumulating matmul
    per p builds s1 = sum_p u_hat directly in PSUM (iter-1 coupling coeffs
    are uniform 1/10).
  routing iters (VectorE/ScalarE): softmax over o, weighted p-reduction,
    squash, b-update — all as [128, 23040] strided mul/reduce ops.
  cross-core: AllReduce (gpsimd collective) of the [128,160] partial s for
    iters 1 and 2; iter 3's partial s is returned and reduced on the host.
"""

import sys

sys.path.insert(0, "/opt/trn_rl_repo")

import numpy as np

B, P, IN_D, O, D = 128, 1152, 8, 10, 16
OD = O * D           # 160
NCORES = 8
PLOC = P // NCORES   # 144
EPS = 1e-8

_CACHE = {}


def _build():
    import os
    from concourse import bass, bacc, tile, mybir

    no_cc = bool(os.environ.get("CAPS_NO_CC"))
    f32 = mybir.dt.float32
    nc = bacc.Bacc("TRN2", target_bir_lowering=False, debug=False,
                   num_devices=1 if no_cc else NCORES)

    xT_d = nc.dram_tensor("xT", [IN_D, PLOC, B], f32, kind="ExternalInput")
    wT_d = nc.dram_tensor("wT", [IN_D, PLOC, OD], f32, kind="ExternalInput")
    out_d = nc.dram_tensor("sp3", [B, OD], f32, kind="ExternalOutput")

    CHUNK = 24            # p's per input-DMA chunk
    EV = 3                # p's per PSUM bank / evacuation copy
    NBLK = PLOC // EV     # 48 evacuation blocks

    with tile.TileContext(nc) as tc:
        with (
            tc.tile_pool(name="persist", bufs=1) as pp,
            tc.tile_pool(name="dram", bufs=2, space="DRAM") as dp,
            tc.tile_pool(name="psum_ub", bufs=6, space="PSUM") as pub,
            tc.tile_pool(name="psum_s1", bufs=1, space="PSUM") as ps1,
        ):
            uhat = pp.tile([B, PLOC * OD], f32)       # 90 KB/partition
            uhat_f = uhat[:]
            uhat4 = uhat_f.rearrange("b (p o d) -> b p o d", p=PLOC, o=O, d=D)

            s1_ps = ps1.tile([B, OD], f32)

            # ---------------- phase 1: u_hat + s1 ----------------
            with tc.tile_pool(name="p1", bufs=2) as p1:
                for ch in range(PLOC // CHUNK):
                    xc = p1.tile([IN_D, CHUNK, B], f32, tag="xc")
                    wc = p1.tile([IN_D, CHUNK, OD], f32, tag="wc")
                    sl = slice(ch * CHUNK, (ch + 1) * CHUNK)
                    nc.sync.dma_start(xc[:], xT_d[:, sl, :])
                    nc.sync.dma_start(wc[:], wT_d[:, sl, :])
                    for blk in range(CHUNK // EV):
                        ub = pub.tile([B, EV * OD], f32, tag="ub")
                        for k in range(EV):
                            j = blk * EV + k
                            p_glob = ch * CHUNK + j
                            nc.tensor.matmul(
                                ub[:, k * OD:(k + 1) * OD],
                                xc[:, j, :], wc[:, j, :],
                                start=True, stop=True,
                            )
                            nc.tensor.matmul(
                                s1_ps[:], xc[:, j, :], wc[:, j, :],
                                start=(p_glob == 0), stop=(p_glob == PLOC - 1),
                                skip_group_check=True,
                            )
                        gblk = ch * (CHUNK // EV) + blk
                        dst = uhat_f[:, gblk * EV * OD:(gblk + 1) * EV * OD]
                        if gblk % 2 == 0:
                            nc.scalar.copy(dst, ub[:])
                        else:
                            nc.vector.tensor_copy(dst, ub[:])

            with tc.tile_pool(name="work", bufs=1) as wp:
                # ---------------- routing tiles ----------------
                tmp = wp.tile([B, PLOC * OD], f32)        # 90 KB/partition
                tmp4 = tmp[:].rearrange("b (p o d) -> b p o d", p=PLOC, o=O, d=D)
                tmp_pod = tmp[:].rearrange("b (p o d) -> b o d p", p=PLOC, o=O, d=D)

                b_route = wp.tile([B, PLOC * O], f32)
                br3 = b_route[:].rearrange("b (p o) -> b p o", p=PLOC, o=O)
                delta = wp.tile([B, PLOC * O], f32)
                eb = wp.tile([B, PLOC * O], f32)
                eb3 = eb[:].rearrange("b (p o) -> b p o", p=PLOC, o=O)
                # delta doubles as the exp-output buffer: softmax's use of it is
                # dead by the time bupd's reduce writes it, and vice versa.
                c_t = wp.tile([B, PLOC * O], f32)
                c3 = c_t[:].rearrange("b (p o) -> b p o", p=PLOC, o=O)
                mx = wp.tile([B, PLOC], f32)
                zs = wp.tile([B, PLOC], f32)
                rz = wp.tile([B, PLOC], f32)

                s_sb = wp.tile([B, OD], f32)
                s_full = wp.tile([B, OD], f32)
                sq = wp.tile([B, OD], f32)
                v_t = wp.tile([B, OD], f32)
                n2 = wp.tile([B, O], f32)
                rt = wp.tile([B, O], f32)
                a1 = wp.tile([B, O], f32)
                a2 = wp.tile([B, O], f32)
                den = wp.tile([B, O], f32)
                rec = wp.tile([B, O], f32)
                g_t = wp.tile([B, O], f32)

                AX = mybir.AxisListType.X

                def bcast(a, b_ap):
                    return bass.broadcast_tensor_aps(a, b_ap)

                def allreduce(src_ap, dst_ap):
                    if no_cc:
                        nc.vector.tensor_copy(dst_ap, src_ap)
                        return
                    cin = dp.tile([B, OD], f32, tag="cin")
                    cout = dp.tile([B, OD], f32, tag="cout", addr_space="Shared")
                    nc.sync.dma_start(cin[:], src_ap)
                    nc.gpsimd.collective_compute(
                        "AllReduce", mybir.AluOpType.add,
                        replica_groups=[list(range(NCORES))],
                        ins=[cin.opt()], outs=[cout.opt()],
                    )
                    nc.sync.dma_start(dst_ap, cout[:])

                def squash():
                    # v = (n2/(1+n2)) * s / (sqrt(n2)+eps), per (b, o)
                    nc.vector.tensor_mul(sq[:], s_full[:], s_full[:])
                    nc.vector.reduce_sum(
                        n2[:], sq[:].rearrange("b (o d) -> b o d", o=O, d=D), axis=AX)
                    nc.scalar.sqrt(rt[:], n2[:])
                    nc.vector.tensor_scalar_add(a1[:], n2[:], 1.0)
                    nc.vector.tensor_scalar_add(a2[:], rt[:], EPS)
                    nc.vector.tensor_mul(den[:], a1[:], a2[:])
                    nc.vector.reciprocal(rec[:], den[:])
                    nc.vector.tensor_mul(g_t[:], n2[:], rec[:])
                    sf3 = s_full[:].rearrange("b (o d) -> b o d", o=O, d=D)
                    v3 = v_t[:].rearrange("b (o d) -> b o d", o=O, d=D)
                    ga, gb = bcast(sf3, g_t[:].unsqueeze(-1))
                    nc.vector.tensor_mul(v3, ga, gb)

                PSPL = 96    # DVE takes p<PSPL, GPSIMD the rest

                def bupd(first):
                    # b_route += sum_d u_hat * v
                    va = v_t[:].rearrange("b (o d) -> b o d", o=O, d=D).unsqueeze(1)
                    ua, vb = bcast(uhat4, va)
                    nc.vector.tensor_mul(tmp4[:, :PSPL], ua[:, :PSPL], vb[:, :PSPL])
                    nc.gpsimd.tensor_mul(tmp4[:, PSPL:], ua[:, PSPL:], vb[:, PSPL:])
                    if first:
                        nc.vector.reduce_sum(b_route[:], tmp4, axis=AX)
                    else:
                        nc.vector.reduce_sum(delta[:], tmp4, axis=AX)
                        nc.vector.tensor_add(b_route[:], b_route[:], delta[:])

                def softmax():
                    # no max-subtraction: |b_route| is small enough that exp()
                    # cannot overflow fp32, and softmax is shift-invariant
                    nc.scalar.activation(delta[:], b_route[:],
                                         mybir.ActivationFunctionType.Exp)
                    d3 = delta[:].rearrange("b (p o) -> b p o", p=PLOC, o=O)
                    nc.vector.reduce_sum(zs[:], d3, axis=AX)
                    nc.vector.reciprocal(rz[:], zs[:])
                    ea, rb = bcast(d3, rz[:].unsqueeze(-1))
                    nc.vector.tensor_mul(c3, ea, rb)

                def weighted_s(dst_ap):
                    # dst = sum_p c * u_hat   (c broadcast over d)
                    ca = c3.unsqueeze(-1)
                    ua, cb = bcast(uhat4, ca)
                    nc.vector.tensor_mul(tmp4[:, :PSPL], ua[:, :PSPL], cb[:, :PSPL])
                    nc.gpsimd.tensor_mul(tmp4[:, PSPL:], ua[:, PSPL:], cb[:, PSPL:])
                    nc.vector.reduce_sum(
                        dst_ap.rearrange("b (o d) -> b o d", o=O, d=D),
                        tmp_pod, axis=AX)

                # ---------------- routing ----------------
                # iter 1: c uniform = 1/10
                nc.scalar.mul(s_sb[:], s1_ps[:], 0.1)
                allreduce(s_sb[:], s_full[:])
                squash()
                bupd(first=True)

                # iter 2
                softmax()
                weighted_s(s_sb[:])
                allreduce(s_sb[:], s_full[:])
                squash()
                bupd(first=False)

                # iter 3: partial s only; reduce + squash on host
                softmax()
                weighted_s(s_sb[:])
                nc.sync.dma_start(out_d[:], s_sb[:])

    nc.compile()
    return nc


def _get_nc():
    if "nc" not in _CACHE:
        _CACHE["nc"] = _build()
    return _CACHE["nc"]


def kernel(x: np.ndarray, W: np.ndarray) -> np.ndarray:
    import os
    from concourse.bass_utils import run_bass_kernel_spmd

    nc = _get_nc()
    trace = bool(os.environ.get("CAPS_TRACE"))
    x = np.ascontiguousarray(x, dtype=np.float32)
    W = np.ascontiguousarray(W, dtype=np.float32)

    in_maps = []
    for c in range(NCORES):
        sl = slice(c * PLOC, (c + 1) * PLOC)
        xT = np.ascontiguousarray(x[:, sl, :].transpose(2, 1, 0))      # [8,144,128]
        wT = np.ascontiguousarray(
            W[0, sl].reshape(PLOC, OD, IN_D).transpose(2, 0, 1))       # [8,144,160]
        in_maps.append({"xT": xT, "wT": wT})

    res = run_bass_kernel_spmd(nc, in_maps, list(range(NCORES)),
                               trace=trace,
                               tmpdir=os.environ.get("CAPS_TRACE_DIR"))
    if trace:
        print(f"HW exec time: {res.exec_time_ns} ns")
        _CACHE["last_result"] = res
    s = np.zeros((B, OD), dtype=np.float32)
    for c in range(NCORES):
        s += res.results[c]["sp3"]

    s = s.reshape(B, O, D)
    n2 = np.sum(s * s, axis=-1, keepdims=True, dtype=np.float32)
    norm = np.sqrt(n2)
    v = (n2 / (1.0 + n2)) * s / (norm + EPS)
    return v.astype(np.float32)



# revision 2
# speedup vs baseline: 2.1918x; 2.1918x over previous
"""DigitCapsuleLayer (dynamic routing) Trainium2 Bass kernel.

Sharding: P-parallel — the 1152 primary capsules are split 144-per-core
across 8 cores; every core holds the full batch B=128 on SBUF partitions.

Per core (bf16 compute):
  phase 1 (TensorE): 12 groups of 12 p's. Per group one K=96 stationary
    load of x (k = 8*p_loc + i) serves 4 matmuls of N=480 against a
    block-diagonal W (prepared host-side, output columns in (d,o) order)
    plus one N=160 dense-stack matmul that accumulates s1 = sum_p u_hat
    directly in PSUM (iter-1 coupling coeffs are uniform 1/10).  PSUM is
    evacuated with contiguous fp32->bf16 copies into an SBUF-resident
    u_hat laid out [b, p, d, o].
  routing iters (VectorE bf16, innermost-contiguous so the DVE runs in
    its fast mode): softmax over o, weighted p-reduction and the b-update
    d-reduction as tree-halving adds, squash on small [128,160] tiles.
  cross-core: AllReduce (gpsimd collective) of the [128,160] partial s
    for iters 1 and 2 (plus a tiny warmup AllReduce overlapped with
    phase 1); iter 3's partial s is returned and reduced on the host.
"""

import sys

sys.path.insert(0, "/opt/trn_rl_repo")

import numpy as np

B, P, IN_D, O, D = 128, 1152, 8, 10, 16
OD = O * D           # 160
NCORES = 8
PLOC = P // NCORES   # 144
G = 12               # p's per matmul group
NG = PLOC // G       # 12 groups
K = G * IN_D         # 96 contraction rows per group
NCH = 4              # 480-column chunks per group
CH = G * OD // NCH   # 480
WCOLS = G * OD + OD  # 1920 blockdiag + 160 dense = 2080
EPS = 1e-8

_CACHE = {}


def _build():
    from concourse import bass, bacc, tile, mybir

    f32 = mybir.dt.float32
    bf16 = mybir.dt.bfloat16
    nc = bacc.Bacc("TRN2", target_bir_lowering=False, debug=False,
                   num_devices=NCORES)

    xg_d = nc.dram_tensor("xg", [K, NG * B], bf16, kind="ExternalInput")
    wg_d = nc.dram_tensor("wg", [NG, K, WCOLS], bf16, kind="ExternalInput")
    out_d = nc.dram_tensor("sp3", [B, OD], f32, kind="ExternalOutput")

    AX = mybir.AxisListType.X

    with tile.TileContext(nc) as tc:
        with (
            tc.tile_pool(name="persist", bufs=1) as pp,
            tc.tile_pool(name="dram", bufs=2, space="DRAM") as dp,
            tc.tile_pool(name="psum_ub", bufs=6, space="PSUM") as pub,
            tc.tile_pool(name="psum_s1", bufs=1, space="PSUM") as ps1,
            tc.tile_pool(name="wpool", bufs=3) as wp_,
            tc.tile_pool(name="work", bufs=1) as wp,
        ):
            uhat = pp.tile([B, PLOC * OD], bf16)      # 45 KB/partition
            uhat_f = uhat[:]
            # memory order is (p, d, o)
            uhat4 = uhat_f.rearrange("b (p d o) -> b p d o", p=PLOC, d=D, o=O)

            s1_ps = ps1.tile([B, OD], f32)            # (d,o) layout
            xa = pp.tile([K, NG * B], bf16)

            def bcast(a, b_ap):
                return bass.broadcast_tensor_aps(a, b_ap)

            # -------- warmup collective (absorbs cc-stream init) --------
            wu_in = dp.tile([B, 1], f32, tag="wu_in")
            wu_out = dp.tile([B, 1], f32, tag="wu_out", addr_space="Shared")
            nc.gpsimd.collective_compute(
                "AllReduce", mybir.AluOpType.add,
                replica_groups=[list(range(NCORES))],
                ins=[wu_in.opt()], outs=[wu_out.opt()],
            )

            # ---------------- phase 1: u_hat + s1 ----------------
            nc.sync.dma_start(xa[:], xg_d[:])
            for g in range(NG):
                wt = wp_.tile([K, WCOLS], bf16, tag="wt")
                nc.sync.dma_start(wt[:], wg_d[g])
                lhsT = xa[:, g * B:(g + 1) * B]
                for q in range(NCH):
                    ub = pub.tile([B, CH], f32, tag="ub")
                    nc.tensor.matmul(
                        ub[:], lhsT, wt[:, q * CH:(q + 1) * CH],
                        start=True, stop=True,
                    )
                    blk = g * NCH + q
                    dst = uhat_f[:, blk * CH:(blk + 1) * CH]
                    if q % 2 == 0:
                        nc.scalar.copy(dst, ub[:])
                    else:
                        nc.vector.tensor_copy(dst, ub[:])
                nc.tensor.matmul(
                    s1_ps[:], lhsT, wt[:, G * OD:],
                    start=(g == 0), stop=(g == NG - 1),
                    skip_group_check=True,
                )

            # ---------------- routing tiles ----------------
            tmp = wp.tile([B, PLOC * OD], bf16)       # 45 KB/partition
            tmp_f = tmp[:]
            tmp4 = tmp_f.rearrange("b (p d o) -> b p d o", p=PLOC, d=D, o=O)

            b_route = wp.tile([B, PLOC * O], f32)
            br3 = b_route[:].rearrange("b (p o) -> b p o", p=PLOC, o=O)
            delta = wp.tile([B, PLOC * O], f32)
            d3 = delta[:].rearrange("b (p o) -> b p o", p=PLOC, o=O)
            e_t = wp.tile([B, PLOC * O], bf16)
            e3 = e_t[:].rearrange("b (p o) -> b p o", p=PLOC, o=O)
            c_t = wp.tile([B, PLOC * O], bf16)
            c3 = c_t[:].rearrange("b (p o) -> b p o", p=PLOC, o=O)
            zs = wp.tile([B, PLOC], f32)
            rz = wp.tile([B, PLOC], f32)

            s_sb = wp.tile([B, OD], f32)              # (d,o) layout
            s_full = wp.tile([B, OD], f32)
            sq = wp.tile([B, OD], f32)
            v_t = wp.tile([B, OD], bf16)              # (d,o) layout
            n2 = wp.tile([B, O], f32)
            rt = wp.tile([B, O], f32)
            a1 = wp.tile([B, O], f32)
            a2 = wp.tile([B, O], f32)
            den = wp.tile([B, O], f32)
            rec = wp.tile([B, O], f32)
            g_t = wp.tile([B, O], f32)

            def allreduce(src_ap, dst_ap):
                cin = dp.tile([B, OD], f32, tag="cin")
                cout = dp.tile([B, OD], f32, tag="cout", addr_space="Shared")
                nc.sync.dma_start(cin[:], src_ap)
                nc.gpsimd.collective_compute(
                    "AllReduce", mybir.AluOpType.add,
                    replica_groups=[list(range(NCORES))],
                    ins=[cin.opt()], outs=[cout.opt()],
                )
                nc.sync.dma_start(dst_ap, cout[:])

            def squash():
                # v = (n2/(1+n2)) * s / (sqrt(n2)+eps), per (b, o)
                nc.vector.tensor_mul(sq[:], s_full[:], s_full[:])
                nc.vector.reduce_sum(
                    n2[:], sq[:].rearrange("b (d o) -> b o d", d=D, o=O),
                    axis=AX)
                nc.scalar.sqrt(rt[:], n2[:])
                nc.vector.tensor_scalar_add(a1[:], n2[:], 1.0)
                nc.vector.tensor_scalar_add(a2[:], rt[:], EPS)
                nc.vector.tensor_mul(den[:], a1[:], a2[:])
                nc.vector.reciprocal(rec[:], den[:])
                nc.vector.tensor_mul(g_t[:], n2[:], rec[:])
                sf3 = s_full[:].rearrange("b (d o) -> b d o", d=D, o=O)
                v3 = v_t[:].rearrange("b (d o) -> b d o", d=D, o=O)
                ga, gb = bcast(sf3, g_t[:].unsqueeze(1))
                nc.vector.tensor_mul(v3, ga, gb)

            def bupd(first):
                # b_route += sum_d u_hat * v   (tree-halve over d)
                va = v_t[:].rearrange("b (d o) -> b d o", d=D, o=O).unsqueeze(1)
                ua, vb = bcast(uhat4, va)
                nc.vector.tensor_mul(tmp4, ua, vb)
                t = tmp4
                nc.vector.tensor_add(t[:, :, 0:8], t[:, :, 0:8], t[:, :, 8:16])
                nc.vector.tensor_add(t[:, :, 0:4], t[:, :, 0:4], t[:, :, 4:8])
                nc.vector.tensor_add(t[:, :, 0:2], t[:, :, 0:2], t[:, :, 2:4])
                if first:
                    nc.vector.tensor_add(br3, t[:, :, 0], t[:, :, 1])
                else:
                    nc.vector.tensor_add(d3, t[:, :, 0], t[:, :, 1])
                    nc.vector.tensor_add(b_route[:], b_route[:], delta[:])

            def softmax():
                # no max-subtraction: |b_route| is small enough that exp()
                # cannot overflow, and softmax is shift-invariant
                nc.scalar.activation(e_t[:], b_route[:],
                                     mybir.ActivationFunctionType.Exp)
                nc.vector.reduce_sum(zs[:], e3, axis=AX)
                nc.vector.reciprocal(rz[:], zs[:])
                ea, rb = bcast(e3, rz[:].unsqueeze(-1))
                nc.vector.tensor_mul(c3, ea, rb)

            def weighted_s():
                # s_sb = sum_p c * u_hat   (tree-halve over p)
                ca = c3.unsqueeze(2)
                ua, cb = bcast(uhat4, ca)
                nc.vector.tensor_mul(tmp4, ua, cb)
                tf = tmp_f
                m = OD
                nc.vector.tensor_add(tf[:, :72 * m], tf[:, :72 * m],
                                     tf[:, 72 * m:144 * m])
                nc.vector.tensor_add(tf[:, :36 * m], tf[:, :36 * m],
                                     tf[:, 36 * m:72 * m])
                nc.vector.tensor_add(tf[:, :18 * m], tf[:, :18 * m],
                                     tf[:, 18 * m:36 * m])
                nc.vector.tensor_add(tf[:, :9 * m], tf[:, :9 * m],
                                     tf[:, 9 * m:18 * m])
                nc.vector.tensor_add(tf[:, :4 * m], tf[:, :4 * m],
                                     tf[:, 4 * m:8 * m])
                nc.vector.tensor_add(tf[:, :2 * m], tf[:, :2 * m],
                                     tf[:, 2 * m:4 * m])
                nc.vector.tensor_add(tf[:, :m], tf[:, :m], tf[:, m:2 * m])
                nc.vector.tensor_add(s_sb[:], tf[:, :m], tf[:, 8 * m:9 * m])

            # ---------------- routing ----------------
            # iter 1: c uniform = 1/10
            nc.scalar.mul(s_sb[:], s1_ps[:], 0.1)
            allreduce(s_sb[:], s_full[:])
            squash()
            bupd(first=True)

            # iter 2
            softmax()
            weighted_s()
            allreduce(s_sb[:], s_full[:])
            squash()
            bupd(first=False)

            # iter 3: partial s only; reduce + squash on host
            softmax()
            weighted_s()
            nc.sync.dma_start(out_d[:], s_sb[:])

    nc.compile()
    return nc


def _get_nc():
    if "nc" not in _CACHE:
        _CACHE["nc"] = _build()
    return _CACHE["nc"]


def kernel(x: np.ndarray, W: np.ndarray) -> np.ndarray:
    import os
    import ml_dtypes
    from concourse.bass_utils import run_bass_kernel_spmd

    nc = _get_nc()
    trace = bool(os.environ.get("CAPS_TRACE"))
    x = np.ascontiguousarray(x, dtype=np.float32)
    W = np.ascontiguousarray(W, dtype=np.float32)
    bf = ml_dtypes.bfloat16

    in_maps = []
    for c in range(NCORES):
        sl = slice(c * PLOC, (c + 1) * PLOC)
        xc = x[:, sl, :]                              # [B, 144, 8]
        # xg[k=8*pl+i, g*B+b] = x[b, 12g+pl, i]
        xg = (xc.transpose(1, 2, 0)                   # [p, i, b]
              .reshape(NG, K, B)                      # [g, (pl i), b]
              .transpose(1, 0, 2)                     # [k, g, b]
              .reshape(K, NG * B))
        # W blocks in (d,o) column order: wb[g, pl, i, 10d+o] = W[p,o,d,i]
        wb = (W[0, sl]                                # [144, 10, 16, 8]
              .transpose(0, 3, 2, 1)                  # [p, i, d, o]
              .reshape(NG, G, IN_D, D * O))
        wg = np.zeros((NG, K, WCOLS), dtype=np.float32)
        for pl in range(G):
            wg[:, 8 * pl:8 * pl + 8, OD * pl:OD * (pl + 1)] = wb[:, pl]
            wg[:, 8 * pl:8 * pl + 8, G * OD:] = wb[:, pl]
        in_maps.append({"xg": xg.astype(bf), "wg": wg.astype(bf)})

    res = run_bass_kernel_spmd(nc, in_maps, list(range(NCORES)),
                               trace=trace,
                               tmpdir=os.environ.get("CAPS_TRACE_DIR"))
    if trace:
        print(f"HW exec time: {res.exec_time_ns} ns")
        _CACHE["last_result"] = res
    s = np.zeros((B, OD), dtype=np.float32)
    for c in range(NCORES):
        s += res.results[c]["sp3"]

    s = s.reshape(B, D, O).transpose(0, 2, 1)         # (d,o) -> [B, O, D]
    n2 = np.sum(s * s, axis=-1, keepdims=True, dtype=np.float32)
    norm = np.sqrt(n2)
    v = (n2 / (1.0 + n2)) * s / (norm + EPS)
    return v.astype(np.float32)


# revision 11
# speedup vs baseline: 2.4509x; 1.1182x over previous
"""DigitCapsuleLayer (dynamic routing) Trainium2 Bass kernel.

Sharding: P-parallel — the 1152 primary capsules are split 144-per-core
across 8 cores; every core holds the full batch B=128 on SBUF partitions.

Per core (bf16 compute):
  phase 1 (TensorE): 24 groups of 6 p's. Per group one K=48 stationary
    load of x (k = 8*p_loc + i) serves 2 matmuls of N=480 against a
    block-diagonal W (prepared host-side, output columns in (d,o) order).
    W DMAs alternate between the SP and Activation HWDGE queues.  PSUM is
    evacuated with contiguous fp32->bf16 copies into an SBUF-resident
    u_hat laid out [b, p, d, o].
  s1 = sum over ALL 1152 p of u_hat is computed redundantly on every
    core as 72 accumulating K=128 matmuls against replicated dense-stack
    inputs (~11us of PE time) — this removes iteration 1's AllReduce, so
    the collectives stream's ~75us init cost runs concurrently with the
    first ~80us of routing compute instead of stalling it.
  routing iters (VectorE bf16): softmax over o; the b-update uses the
    identity sum_d u*(g*s) = g*sum_d(u*s) so the squash-coefficient g
    (the only sqrt) is computed off the critical path while the d-tree
    runs; a dummy exp preloads the ACT exp table off-critical.  All
    reductions are tree-halving adds (DVE runs 2-byte ops at 2x).
  cross-core: a single AllReduce (gpsimd collective) of the [128,160]
    partial s for iter 2 (plus a tiny warmup AllReduce that absorbs the
    first-collective cost); iter 3's partial s is returned and reduced
    on the host.
"""

import sys

sys.path.insert(0, "/opt/trn_rl_repo")

import numpy as np

B, P, IN_D, O, D = 128, 1152, 8, 10, 16
OD = O * D           # 160
NCORES = 8
PLOC = P // NCORES   # 144
G = 6                # p's per matmul group
NG = PLOC // G       # 24 groups
K = G * IN_D         # 48 contraction rows per group
NCH = 2              # 480-column chunks per group
CH = G * OD // NCH   # 480
WCOLS = G * OD       # 960 blockdiag columns
NT = P * IN_D // 128  # 72 K-tiles for the full-s1 matmul
EPS = 1e-8

_CACHE = {}


def _build():
    from concourse import bass, bacc, tile, mybir

    f32 = mybir.dt.float32
    bf16 = mybir.dt.bfloat16
    nc = bacc.Bacc("TRN2", target_bir_lowering=False, debug=False,
                   num_devices=NCORES)

    xg_d = nc.dram_tensor("xg", [K, NG * B], bf16, kind="ExternalInput")
    wg_d = nc.dram_tensor("wg", [NG, K, WCOLS], bf16, kind="ExternalInput")
    xf_d = nc.dram_tensor("xf", [128, NT * 128], bf16, kind="ExternalInput")
    wf_d = nc.dram_tensor("wf", [128, NT * OD], bf16, kind="ExternalInput")
    out_d = nc.dram_tensor("sp3", [B, OD], f32, kind="ExternalOutput")

    AX = mybir.AxisListType.X

    with tile.TileContext(nc) as tc:
        with (
            tc.tile_pool(name="persist", bufs=1) as pp,
            tc.tile_pool(name="dram", bufs=2, space="DRAM") as dp,
            tc.tile_pool(name="psum_ub", bufs=6, space="PSUM") as pub,
            tc.tile_pool(name="psum_s1", bufs=1, space="PSUM") as ps1,
            tc.tile_pool(name="wpool", bufs=4) as wp_,
            tc.tile_pool(name="work", bufs=1) as wp,
        ):
            uhat = pp.tile([B, PLOC * OD], bf16)      # 45 KB/partition
            uhat_f = uhat[:]
            # memory order is (p, d, o)
            uhat4 = uhat_f.rearrange("b (p d o) -> b p d o", p=PLOC, d=D, o=O)

            s1_ps = ps1.tile([B, OD], f32)            # (d,o) layout
            xa = pp.tile([K, NG * B], bf16)
            xfull = pp.tile([128, NT * 128], bf16)    # 18 KB/partition
            wfull = pp.tile([128, NT * OD], bf16)     # 22.5 KB/partition

            def bcast(a, b_ap):
                return bass.broadcast_tensor_aps(a, b_ap)

            # -------- warmup collective (absorbs cc-stream init) --------
            wu_in = dp.tile([B, 1], f32, tag="wu_in")
            wu_out = dp.tile([B, 1], f32, tag="wu_out", addr_space="Shared")
            nc.gpsimd.collective_compute(
                "AllReduce", mybir.AluOpType.add,
                replica_groups=[list(range(NCORES))],
                ins=[wu_in.opt()], outs=[wu_out.opt()],
            )

            # ---------------- phase 1: u_hat + full s1 ----------------
            nc.sync.dma_start(xa[:], xg_d[:])
            nc.sync.dma_start(xfull[:], xf_d[:])
            nc.scalar.dma_start(wfull[:], wf_d[:])

            # s1-tile schedule: none for the first 8 groups (their PE time
            # covers the xfull/wfull DMA), then 4/5 per group
            s1_sched = [0] * 8 + [4] * 8 + [5] * 8
            assert sum(s1_sched) == NT
            t_done = 0

            for g in range(NG):
                wt = wp_.tile([K, WCOLS], bf16, tag="wt")
                if g % 2 == 0:
                    nc.scalar.dma_start(wt[:], wg_d[g])
                else:
                    nc.sync.dma_start(wt[:], wg_d[g])
                lhsT = xa[:, g * B:(g + 1) * B]
                for q in range(NCH):
                    ub = pub.tile([B, CH], f32, tag="ub")
                    nc.tensor.matmul(
                        ub[:], lhsT, wt[:, q * CH:(q + 1) * CH],
                        start=True, stop=True,
                    )
                    blk = g * NCH + q
                    dst = uhat_f[:, blk * CH:(blk + 1) * CH]
                    if q % 2 == 0:
                        nc.scalar.copy(dst, ub[:])
                    else:
                        nc.vector.tensor_copy(dst, ub[:])
                for _ in range(s1_sched[g]):
                    t = t_done
                    nc.tensor.matmul(
                        s1_ps[:], xfull[:, t * 128:(t + 1) * 128],
                        wfull[:, t * OD:(t + 1) * OD],
                        start=(t == 0), stop=(t == NT - 1),
                        skip_group_check=True,
                    )
                    t_done += 1

            # ---------------- routing tiles ----------------
            tmp = wp.tile([B, PLOC * OD], bf16)       # 45 KB/partition
            tmp_f = tmp[:]
            tmp4 = tmp_f.rearrange("b (p d o) -> b p d o", p=PLOC, d=D, o=O)

            b_route = wp.tile([B, PLOC * O], f32)
            br3 = b_route[:].rearrange("b (p o) -> b p o", p=PLOC, o=O)
            delta = wp.tile([B, PLOC * O], f32)
            d3 = delta[:].rearrange("b (p o) -> b p o", p=PLOC, o=O)
            dg = wp.tile([B, PLOC * O], f32)
            dg3 = dg[:].rearrange("b (p o) -> b p o", p=PLOC, o=O)
            e_t = wp.tile([B, PLOC * O], bf16)
            e3 = e_t[:].rearrange("b (p o) -> b p o", p=PLOC, o=O)
            c_t = wp.tile([B, PLOC * O], bf16)
            c3 = c_t[:].rearrange("b (p o) -> b p o", p=PLOC, o=O)
            zs = wp.tile([B, PLOC], f32)
            rz = wp.tile([B, PLOC], f32)

            s_bf = wp.tile([B, OD], bf16)             # summed s, (d,o)
            s_sb = wp.tile([B, OD], f32)
            s_full = wp.tile([B, OD], f32)
            sq = wp.tile([B, OD], f32)
            s_out = wp.tile([B, OD], f32)
            n2 = wp.tile([B, O], f32)
            rt = wp.tile([B, O], f32)
            a1 = wp.tile([B, O], f32)
            rec = wp.tile([B, O], f32)
            g_t = wp.tile([B, O], f32)
            dume = wp.tile([B, 1], f32)

            def allreduce(src_ap, dst_ap):
                cin = dp.tile([B, OD], f32, tag="cin")
                cout = dp.tile([B, OD], f32, tag="cout", addr_space="Shared")
                nc.sync.dma_start(cin[:], src_ap)
                nc.gpsimd.collective_compute(
                    "AllReduce", mybir.AluOpType.add,
                    replica_groups=[list(range(NCORES))],
                    ins=[cin.opt()], outs=[cout.opt()],
                )
                nc.sync.dma_start(dst_ap, cout[:])

            def coeff_g():
                # g = sqrt(n2) / (1 + n2), per (b, o); off critical path.
                # the trailing dummy exp preloads the ACT exp table so the
                # next softmax doesn't pay the table switch
                nc.vector.tensor_mul(sq[:], s_full[:], s_full[:])
                nc.vector.reduce_sum(
                    n2[:], sq[:].rearrange("b (d o) -> b o d", d=D, o=O),
                    axis=AX)
                nc.scalar.sqrt(rt[:], n2[:])
                nc.scalar.activation(dume[:], dume[:],
                                     mybir.ActivationFunctionType.Exp)
                nc.vector.tensor_scalar_add(a1[:], n2[:], 1.0)
                nc.vector.reciprocal(rec[:], a1[:])
                nc.vector.tensor_mul(g_t[:], rt[:], rec[:])

            def bupd(first):
                # b_route += g * (sum_d u_hat * s)   (tree-halve over d)
                sa = s_bf[:].rearrange("b (d o) -> b d o", d=D, o=O).unsqueeze(1)
                ua, sb = bcast(uhat4, sa)
                nc.vector.tensor_mul(tmp4, ua, sb)
                coeff_g()
                t = tmp4
                nc.vector.tensor_add(t[:, :, 0:8], t[:, :, 0:8], t[:, :, 8:16])
                nc.vector.tensor_add(t[:, :, 0:4], t[:, :, 0:4], t[:, :, 4:8])
                nc.vector.tensor_add(t[:, :, 0:2], t[:, :, 0:2], t[:, :, 2:4])
                nc.vector.tensor_add(d3, t[:, :, 0], t[:, :, 1])
                da, gb = bcast(d3, g_t[:].unsqueeze(1))
                if first:
                    nc.vector.tensor_mul(br3, da, gb)
                else:
                    nc.vector.tensor_mul(dg3, da, gb)
                    nc.vector.tensor_add(b_route[:], b_route[:], dg[:])

            def softmax():
                # no max-subtraction: |b_route| is small enough that exp()
                # cannot overflow, and softmax is shift-invariant
                nc.scalar.activation(e_t[:], b_route[:],
                                     mybir.ActivationFunctionType.Exp)
                nc.vector.reduce_sum(zs[:], e3, axis=AX)
                nc.vector.reciprocal(rz[:], zs[:])
                ea, rb = bcast(e3, rz[:].unsqueeze(-1))
                nc.vector.tensor_mul(c3, ea, rb)

            def weighted_s(dst_ap):
                # dst = sum_p c * u_hat   (tree-halve over p)
                ca = c3.unsqueeze(2)
                ua, cb = bcast(uhat4, ca)
                nc.vector.tensor_mul(tmp4, ua, cb)
                tf = tmp_f
                m = OD
                nc.vector.tensor_add(tf[:, :72 * m], tf[:, :72 * m],
                                     tf[:, 72 * m:144 * m])
                nc.vector.tensor_add(tf[:, :36 * m], tf[:, :36 * m],
                                     tf[:, 36 * m:72 * m])
                nc.vector.tensor_add(tf[:, :18 * m], tf[:, :18 * m],
                                     tf[:, 18 * m:36 * m])
                nc.vector.tensor_add(tf[:, :9 * m], tf[:, :9 * m],
                                     tf[:, 9 * m:18 * m])
                nc.vector.tensor_add(tf[:, :4 * m], tf[:, :4 * m],
                                     tf[:, 4 * m:8 * m])
                nc.vector.tensor_add(tf[:, :2 * m], tf[:, :2 * m],
                                     tf[:, 2 * m:4 * m])
                nc.vector.tensor_add(tf[:, :m], tf[:, :m], tf[:, m:2 * m])
                nc.vector.tensor_add(dst_ap, tf[:, :m], tf[:, 8 * m:9 * m])

            # ---------------- routing ----------------
            # iter 1: c uniform = 1/10; s1 is already global — no comms
            nc.scalar.mul(s_bf[:], s1_ps[:], 0.1)
            nc.scalar.mul(s_full[:], s1_ps[:], 0.1)
            bupd(first=True)

            # iter 2
            softmax()
            weighted_s(s_sb[:])
            allreduce(s_sb[:], s_full[:])
            nc.scalar.copy(s_bf[:], s_full[:])
            bupd(first=False)

            # iter 3: partial s only; reduce + squash on host
            softmax()
            weighted_s(s_out[:])
            nc.sync.dma_start(out_d[:], s_out[:])

    nc.compile()
    return nc


def _get_nc():
    if "nc" not in _CACHE:
        _CACHE["nc"] = _build()
    return _CACHE["nc"]


def kernel(x: np.ndarray, W: np.ndarray) -> np.ndarray:
    import os
    import ml_dtypes
    from concourse.bass_utils import run_bass_kernel_spmd

    nc = _get_nc()
    trace = bool(os.environ.get("CAPS_TRACE"))
    x = np.ascontiguousarray(x, dtype=np.float32)
    W = np.ascontiguousarray(W, dtype=np.float32)
    bf = ml_dtypes.bfloat16

    # replicated full-s1 operands, (p,i)-major K tiles, (d,o) columns
    xf = (x.transpose(1, 2, 0)                        # [p, i, b]
          .reshape(NT, 128, B)
          .transpose(1, 0, 2)
          .reshape(128, NT * B).astype(bf))
    wf = (W[0].transpose(0, 3, 2, 1)                  # [p, i, d, o]
          .reshape(NT, 128, D * O)
          .transpose(1, 0, 2)
          .reshape(128, NT * OD).astype(bf))

    in_maps = []
    for c in range(NCORES):
        sl = slice(c * PLOC, (c + 1) * PLOC)
        xc = x[:, sl, :]                              # [B, 144, 8]
        # xg[k=8*pl+i, g*B+b] = x[b, G*g+pl, i]
        xg = (xc.transpose(1, 2, 0)                   # [p, i, b]
              .reshape(NG, K, B)                      # [g, (pl i), b]
              .transpose(1, 0, 2)                     # [k, g, b]
              .reshape(K, NG * B))
        # W blocks in (d,o) column order: wb[g, pl, i, 10d+o] = W[p,o,d,i]
        wb = (W[0, sl]                                # [144, 10, 16, 8]
              .transpose(0, 3, 2, 1)                  # [p, i, d, o]
              .reshape(NG, G, IN_D, D * O))
        wg = np.zeros((NG, K, WCOLS), dtype=np.float32)
        for pl in range(G):
            wg[:, 8 * pl:8 * pl + 8, OD * pl:OD * (pl + 1)] = wb[:, pl]
        in_maps.append({"xg": xg.astype(bf), "wg": wg.astype(bf),
                        "xf": xf, "wf": wf})

    res = run_bass_kernel_spmd(nc, in_maps, list(range(NCORES)),
                               trace=trace,
                               tmpdir=os.environ.get("CAPS_TRACE_DIR"))
    if trace:
        print(f"HW exec time: {res.exec_time_ns} ns")
        _CACHE["last_result"] = res
    s = np.zeros((B, OD), dtype=np.float32)
    for c in range(NCORES):
        s += res.results[c]["sp3"]

    s = s.reshape(B, D, O).transpose(0, 2, 1)         # (d,o) -> [B, O, D]
    n2 = np.sum(s * s, axis=-1, keepdims=True, dtype=np.float32)
    norm = np.sqrt(n2)
    v = (n2 / (1.0 + n2)) * s / (norm + EPS)
    return v.astype(np.float32)
